# revision 34
# baseline (speedup 1.0000x reference)
"""Trainium2 Bass kernel for an 8-head self-attention block (MHA).

Problem: x[2, 4096, 512], 8 heads x 64 dims, torch-Linear q/k/v/o projections,
softmax attention, residual:  out = softmax(q k^T / 8) v @ Wo^T + bo + x.

Sharding (8 NeuronCores, no collectives): core c handles batch b = c // 4 and
query rows (c % 4) * 1024 ... + 1024, for ALL heads.  K/V for the full
sequence are computed on every core of a batch group, so the output
projection is fully local to a core.

Engine-level structure (trace-driven; ACT exp is a hard ~300us floor, so the
whole kernel is built to keep ACT saturated while PE stays dense enough to
avoid the HAM half-clock throttle):

  - PV in "transposed" orientation: stationary = [V | 1] chunk [s=128, 65]
    (65-column LDWEIGHTS), moving = P~ [s=128, q=512] -> psum oT[65, q]
    accumulated over the 32 s-chunks; row 64 collects the softmax
    denominator.  No per-qc LDWEIGHTS stream, no PE transposes.
  - phase B: pair (0,1) attention at q=512 granularity, with ALL projection
    matmul groups (K, V, and the deferred 3/4 of Q) interleaved 2-per-chunk
    as PE filler between the exp-gated score matmuls.  Projections get
    their own 2-bank psum ring so they never queue behind an exp.
    Per-chunk PE work (~3.4us) > ACT work (2.9us): PE stays dense -> warm.
  - phase C: pairs (2,3), (4,5), (6,7) with q=1024 score tiles and exps
    (best ACT overhead that fits psum); measured 98% ACT occupancy.
  - normalization: one DVE copy stages psum rows 0:65 (freeing the psum
    accumulator), then gpsimd extracts/broadcasts the denominator row,
    DVE fast-reciprocal + multiply write bf16 oT (cross-partition-base
    DVE writes are legal and verified).
  - a dummy exp at t=0 preloads the ACT spline table during startup DMA;
    the first real exp lands ~10us in (startup emits only the f=0 chunks
    of Q and K that the first score matmul actually needs).
"""

import numpy as np

B = 2
S = 4096
E = 512
H = 8
D = 64
P = 128
EC = E // P          # 4 e-chunks
FC = E // P          # 4 f-chunks
NJ = S // P          # 32 s-chunks
QR = S // 4          # 1024 query rows per core
NQS = QR // 512      # 2 query strips of 512
NKS = S // 512       # 8 s-strips of 512

_CACHE = {}


def _build_nc():
    import concourse.bass as bass
    import concourse.tile as tile
    from concourse import bacc, mybir

    f32 = mybir.dt.float32
    bf16 = mybir.dt.bfloat16
    AFT = mybir.ActivationFunctionType
    Alu = mybir.AluOpType

    nc = bacc.Bacc("TRN2", target_bir_lowering=False, debug=False, num_devices=8)

    xT_d = nc.declare_dram_parameter("xT", [P, EC, S], bf16, isOutput=False)
    xqT_d = nc.declare_dram_parameter("xqT", [P, EC, QR], bf16, isOutput=False)
    xres_d = nc.declare_dram_parameter("xres", [P, QR // P, E], bf16,
                                       isOutput=False)
    ident_d = nc.declare_dram_parameter("ident", [P, P], bf16, isOutput=False)
    wqT_d = nc.declare_dram_parameter("wqT", [P, EC, E], bf16, isOutput=False)
    wkT_d = nc.declare_dram_parameter("wkT", [P, EC, E], bf16, isOutput=False)
    wvT_d = nc.declare_dram_parameter("wvT", [P, EC, E], bf16, isOutput=False)
    woT_d = nc.declare_dram_parameter("woT", [P, EC, E], bf16, isOutput=False)
    bq_d = nc.declare_dram_parameter("bq", [P, FC], f32, isOutput=False)
    bk_d = nc.declare_dram_parameter("bk", [P, FC], f32, isOutput=False)
    bv_d = nc.declare_dram_parameter("bv", [E], f32, isOutput=False)
    out_d = nc.declare_dram_parameter("out", [QR, E], f32, isOutput=True)

    with tile.TileContext(nc) as tc:
        with tc.tile_pool(name="const", bufs=1) as const, \
             tc.tile_pool(name="persist", bufs=1) as persist:

            wo_sb = const.tile([P, EC, E], bf16)
            bq_sb = const.tile([P, FC], f32)
            bk_sb = const.tile([P, FC], f32)
            bv_sb = const.tile([P, E], f32)
            xres_sb = const.tile([P, QR // P, E], bf16)
            ident_sb = const.tile([P, P], bf16)

            kT_sb = persist.tile([P, FC, S], bf16)           # 32 KB/p
            qT_sb = persist.tile([P, FC, QR], bf16)          # 8 KB/p
            v_sb = persist.tile([P, NJ, H, 65], bf16)        # 32.5 KB/p
            oT_sb = persist.tile([P, FC, QR], bf16)          # 8 KB/p

            ones_sb = const.tile([P, 64], f32)
            nc.vector.memset(v_sb[:, :, :, 64:65], 1.0)
            nc.vector.memset(ones_sb[0:1, :], 1.0)

            with tc.tile_pool(name="wpool", bufs=1) as wpool, \
                 tc.tile_pool(name="xtp", bufs=3) as xtp, \
                 tc.tile_pool(name="ptp", bufs=1) as ptp, \
                 tc.tile_pool(name="npool", bufs=1) as npool, \
                 tc.tile_pool(name="opool", bufs=2) as opool, \
                 tc.tile_pool(name="ps_pv", bufs=2, space="PSUM") as ps_pv:

                # dummy exp: preloads the ACT table set during startup DMA
                dum = npool.tile([P, 1], f32, tag="dum", bufs=1)
                nc.vector.memset(dum[:], 0.0)
                nc.scalar.activation(dum[:], dum[:], AFT.Exp)

                wq_sb = wpool.tile([P, EC, E], bf16)
                wk_sb = wpool.tile([P, EC, E], bf16)
                wv_sb = wpool.tile([P, EC, E], bf16)
                xq_t = [xtp.tile([P, EC, 512], bf16, tag="xq", bufs=2,
                                 name=f"xq{i}") for i in range(NQS)]
                # one DMA per tensor (per-DMA overhead ~1.5us dominates the
                # startup critical path at finer granularity)
                nc.sync.dma_start(out=xq_t[0][:], in_=xqT_d[:, :, 0:512])
                nc.sync.dma_start(out=wq_sb[:], in_=wqT_d[:])
                nc.sync.dma_start(out=bq_sb[:], in_=bq_d[:])
                # K path + strips on a second hardware DMA queue, in
                # parallel with the Q path above
                nc.scalar.dma_start(out=wk_sb[:], in_=wkT_d[:])
                nc.scalar.dma_start(out=bk_sb[:], in_=bk_d[:])
                xt_early = {}
                for _s in range(2):
                    _xt = xtp.tile([P, EC, 512], bf16, tag="xt",
                                   name=f"xte{_s}")
                    nc.scalar.dma_start(
                        out=_xt[:],
                        in_=xT_d[:, :, _s * 512:(_s + 1) * 512])
                    xt_early[_s] = _xt
                nc.sync.dma_start(out=xq_t[1][:], in_=xqT_d[:, :, 512:1024])
                nc.scalar.dma_start(out=wv_sb[:], in_=wvT_d[:])
                nc.sync.dma_start(
                    out=bv_sb[:],
                    in_=bass.AP(tensor=bv_d, offset=0, ap=[[0, P], [1, E]]))

                xt_tiles = {}

                def dma_strip(s):
                    if s in xt_early:
                        xt_tiles[s] = xt_early[s]
                        return
                    xt = xtp.tile([P, EC, 512], bf16, tag="xt")
                    ssl = slice(s * 512, (s + 1) * 512)
                    nc.sync.dma_start(out=xt[:], in_=xT_d[:, :, ssl])
                    xt_tiles[s] = xt

                def emit_normalize(pvs, pair, last=False, bc_pool=None):
                    # oT[0:64] = psum_rows_0:63 * (1 / psum_row_64), per head
                    for i, h in enumerate(pair):
                        fc = h // 2
                        fr = (h % 2) * 64
                        den = npool.tile([P, QR], f32, tag="den", bufs=2)
                        if last:
                            # no next pair waits on these psum banks: read
                            # psum directly; cross-base psum->sbuf DVE copy
                            # (proven by the head-7 multiply) extracts the
                            # denominator row in one step
                            nc.vector.tensor_copy(den[0:1, :],
                                                  pvs[i][64:65, 0:QR])
                            stg = pvs[i]
                        else:
                            stg = npool.tile([P, QR], f32, tag="stg", bufs=2)
                            nc.vector.tensor_copy(stg[0:65, :],
                                                  pvs[i][0:65, 0:QR])
                            nc.vector.tensor_copy(den[0:1, :],
                                                  stg[64:65, :])
                        rc = npool.tile([P, QR], f32, tag="rc", bufs=1)
                        if bc_pool is not None:
                            # tail: K=1 ones-row matmul broadcasts on the idle
                            # PE in ~0.5us (gpsimd bcast measures 1.8-5.2us)
                            bcp = bc_pool.tile([P, QR], f32, tag="sc",
                                               name="bcp")
                            for qs in range(2):
                                qsl = slice(qs * 512, (qs + 1) * 512)
                                nc.tensor.matmul(
                                    bcp[0:64, qsl], ones_sb[0:1, 0:64],
                                    den[0:1, qsl], start=True, stop=True,
                                    skip_group_check=True)
                            nc.vector.reciprocal_approx_fast(
                                rc[0:64, :], bcp[0:64, 0:QR])
                        else:
                            bc = npool.tile([P, QR], f32, tag="bc", bufs=1)
                            nc.gpsimd.partition_broadcast(
                                bc[0:64, :], den[0:1, :], channels=64)
                            nc.vector.reciprocal_approx_fast(
                                rc[0:64, :], bc[0:64, :])
                        nc.vector.tensor_mul(
                            oT_sb[fr:fr + 64, fc, :], stg[0:64, :] if not last
                            else pvs[i][0:64, 0:QR], rc[0:64, :])

                # ---------- phase B ----------
                with tc.tile_pool(name="ps_b", bufs=2, space="PSUM") as ps_b:

                    def emit_pq(f, qs):
                        qsl = slice(qs * 512, (qs + 1) * 512)
                        pq = ps_b.tile([P, 512], f32, tag="pj", name="pq")
                        for e in range(EC):
                            nc.tensor.matmul(
                                pq[:], wq_sb[:, e, f * P:(f + 1) * P],
                                xq_t[qs][:, e, :], start=(e == 0),
                                stop=(e == EC - 1), skip_group_check=True)
                        nc.vector.tensor_scalar(
                            qT_sb[:, f, qsl], pq[:], bq_sb[:, f:f + 1],
                            float(1.0 / np.sqrt(D)), Alu.add, Alu.mult)

                    def emit_pk(s, f):
                        ssl = slice(s * 512, (s + 1) * 512)
                        pk = ps_b.tile([P, 512], f32, tag="pj", name="pk")
                        for e in range(EC):
                            nc.tensor.matmul(
                                pk[:], wk_sb[:, e, f * P:(f + 1) * P],
                                xt_tiles[s][:, e, :], start=(e == 0),
                                stop=(e == EC - 1), skip_group_check=True)
                        nc.vector.tensor_scalar_add(
                            kT_sb[:, f, ssl], pk[:], bk_sb[:, f:f + 1])

                    def emit_v(s, k):
                        j = s * 4 + k
                        pvx = ps_b.tile([P, E], f32, tag="pj", name="pvx")
                        for e in range(EC):
                            nc.tensor.matmul(
                                pvx[:], xt_tiles[s][:, e, k * P:(k + 1) * P],
                                wv_sb[:, e, :], start=(e == 0),
                                stop=(e == EC - 1), skip_group_check=True)
                        pv_v = pvx[:].rearrange("p (h d) -> p h d", h=H)
                        bv_v = bv_sb[:].rearrange("p (h d) -> p h d", h=H)
                        nc.vector.tensor_add(v_sb[:, j, :, 0:64], pv_v[:],
                                             bv_v[:])

                    # filler schedule: (deadline_slot, earliest_slot, fn, args)
                    # pk(s,*) must land inside strip s's xt-tile residency
                    # window [4(s-2), 4s+3] (ring bufs=3); deadlines spread
                    # the per-strip groups ~2 per chunk slot.
                    fillers = []
                    for s in range(1, NKS):
                        fillers.append([4 * s - 1, 4 * (s - 2), emit_pk, (s, 0)])
                    for s in range(NKS):
                        for k in range(4):
                            fillers.append([4 * s + k, 4 * (s - 2), emit_v,
                                            (s, k)])
                        for f in range(1, FC):
                            fillers.append([4 * s + f, 4 * (s - 2), emit_pk,
                                            (s, f)])
                    for f in range(1, FC):
                        for qs in range(NQS):
                            # f=3 Q chunks spill into the bridge's PE slack
                            dl = (NJ + qs) if f == 3 else 8 * f + qs
                            fillers.append([dl, 0, emit_pq, (f, qs)])
                    for fl in fillers:
                        if fl[2] is emit_pk and fl[3] == (NKS - 1, FC - 1):
                            fl[0] = NJ + 2   # pk(7,3) -> bridge slot 2
                    fillers.sort(key=lambda x: x[0])

                    def pop_fillers(j, target=2):
                        n = 0
                        while fillers and fillers[0][0] <= j:
                            _, _, fn, args = fillers.pop(0)
                            fn(*args)
                            n += 1
                        while n < target:
                            idx = next((k for k, fl in enumerate(fillers)
                                        if fl[1] <= j), None)
                            if idx is None:
                                break
                            _, _, fn, args = fillers.pop(idx)
                            fn(*args)
                            n += 1

                    # warm the PE during the startup DMA window: ~26 junk
                    # matmuls on (uninitialized) SBUF, results never read.
                    # HAM needs ~3.4us of sustained PE activity to lift the
                    # half-clock throttle; without this every startup matmul
                    # runs at 1.2 GHz.
                    for w in range(7):
                        jk = ps_b.tile([P, 512], f32, tag="pj", name="jk")
                        nc.tensor.matmul(
                            jk[:], kT_sb[:, 0, 0:P], kT_sb[:, 0, 0:512],
                            start=True, stop=True, skip_group_check=True)

                    # startup compute: just what score chunk j=0 needs
                    dma_strip(0)
                    dma_strip(1)
                    emit_pq(0, 0)
                    emit_pk(0, 0)
                    emit_pq(0, 1)

                    pv01 = [ps_pv.tile([P, QR], f32, tag="pv", name=f"pv0{i}")
                            for i in range(2)]
                    prev_pts = None
                    for j in range(NJ):
                        if j % 4 == 0 and j // 4 + 2 < NKS:
                            dma_strip(j // 4 + 2)
                        pts = {}
                        for qs in range(2):
                            qsl = slice(qs * 512, (qs + 1) * 512)
                            scs = []
                            for i in range(2):
                                fr = i * 64
                                sc = ps_b.tile([P, 512], f32, tag="sc",
                                               name="scb")
                                nc.tensor.matmul(
                                    sc[:],
                                    kT_sb[fr:fr + 64, 0, j * P:(j + 1) * P],
                                    qT_sb[fr:fr + 64, 0, qsl],
                                    start=True, stop=True,
                                    skip_group_check=True)
                                scs.append(sc)
                            for i in range(2):
                                pt = ptp.tile([P, 512], bf16, tag="ptb",
                                              bufs=8)
                                nc.scalar.activation(pt[:], scs[i][:], AFT.Exp)
                                pts[(i, qs)] = pt
                            if qs == 0 and prev_pts is not None:
                                for q2 in range(2):
                                    q2l = slice(q2 * 512, (q2 + 1) * 512)
                                    for i in range(2):
                                        nc.tensor.matmul(
                                            pv01[i][0:65, q2l],
                                            v_sb[:, j - 1, i, :],
                                            prev_pts[(i, q2)][:, :],
                                            start=(j - 1 == 0),
                                            stop=(j - 1 == NJ - 1),
                                            skip_group_check=True)
                        pop_fillers(j)
                        prev_pts = pts
                    held01 = prev_pts
                    pending = (pv01, (0, 1))

                    # bridge: pair (2,3)'s first chunks at q=512 inside the
                    # B psum scope, so neither PE nor ACT idles across the
                    # B->C pool handoff (an idle window here re-throttles
                    # the PE to half clock for the whole next pair).
                    NBR = 3
                    pv23 = [ps_pv.tile([P, QR], f32, tag="pv",
                                       name=f"pv23{i}") for i in range(2)]
                    prev_pts23 = None
                    for j in range(NBR):
                        pts = {}
                        for qs in range(2):
                            qsl = slice(qs * 512, (qs + 1) * 512)
                            scs = []
                            for i in range(2):
                                fr = i * 64
                                sc = ps_b.tile([P, 512], f32, tag="sc",
                                               name="scb2")
                                nc.tensor.matmul(
                                    sc[:],
                                    kT_sb[fr:fr + 64, 1, j * P:(j + 1) * P],
                                    qT_sb[fr:fr + 64, 1, qsl],
                                    start=True, stop=True,
                                    skip_group_check=True)
                                scs.append(sc)
                            for i in range(2):
                                pt = ptp.tile([P, 512], bf16, tag="ptb",
                                              bufs=8)
                                nc.scalar.activation(pt[:], scs[i][:], AFT.Exp)
                                pts[(i, qs)] = pt
                            if qs == 0 and held01 is not None:
                                for q2 in range(2):
                                    q2l = slice(q2 * 512, (q2 + 1) * 512)
                                    for i in range(2):
                                        nc.tensor.matmul(
                                            pv01[i][0:65, q2l],
                                            v_sb[:, NJ - 1, i, :],
                                            held01[(i, q2)][:, :],
                                            start=False, stop=True,
                                            skip_group_check=True)
                                held01 = None
                            if qs == 0 and prev_pts23 is not None:
                                for q2 in range(2):
                                    q2l = slice(q2 * 512, (q2 + 1) * 512)
                                    for i in range(2):
                                        nc.tensor.matmul(
                                            pv23[i][0:65, q2l],
                                            v_sb[:, j - 1, 2 + i, :],
                                            prev_pts23[(i, q2)][:, :],
                                            start=(j - 1 == 0), stop=False,
                                            skip_group_check=True)
                        pop_fillers(NJ + j, target=0)
                        prev_pts23 = pts
                        if j == 1:
                            emit_normalize(*pending)
                            pending = None
                    assert not fillers, [f[0] for f in fillers]

                # tail-only data, off the startup critical path
                nc.sync.dma_start(out=wo_sb[:], in_=woT_d[:])
                nc.sync.dma_start(out=xres_sb[:], in_=xres_d[:])
                nc.sync.dma_start(out=ident_sb[:], in_=ident_d[:])

                # ---------- phase C ----------
                with tc.tile_pool(name="ps_c", bufs=2, space="PSUM") as ps_c:
                    held = None   # previous pair's final PV, deferred so the
                    # next pair's first scores reach the PE queue first and
                    # ACT never gaps across the pair boundary
                    for pi in range(1, 4):
                        pair = (2 * pi, 2 * pi + 1)
                        fc = pi
                        if pi == 1:
                            # resume pair (2,3): bridge already ran j < NBR
                            pvs = pv23
                            prev_pts = prev_pts23
                            jstart = NBR
                        else:
                            pvs = [ps_pv.tile([P, QR], f32, tag="pv",
                                              name=f"pv{pi}{i}")
                                   for i in range(2)]
                            prev_pts = None
                            jstart = 0
                        for j in range(jstart, NJ):
                            jsl = slice(j * P, (j + 1) * P)
                            scs = []
                            for i in range(2):
                                scs.append(ps_c.tile([P, QR], f32, tag="sc",
                                                     name="scc"))
                            for qs in range(2):
                                qsl = slice(qs * 512, (qs + 1) * 512)
                                for i in range(2):
                                    fr = i * 64
                                    nc.tensor.matmul(
                                        scs[i][:, qsl],
                                        kT_sb[fr:fr + 64, fc, jsl],
                                        qT_sb[fr:fr + 64, fc, qsl],
                                        start=True, stop=True,
                                        skip_group_check=True)
                            pts = {}
                            for i in range(2):
                                pt = ptp.tile([P, QR], bf16, tag="ptc", bufs=5)
                                nc.scalar.activation(pt[:], scs[i][:], AFT.Exp)
                                pts[(i,)] = pt
                            if held is not None:
                                hpvs, hpair, hpts = held
                                for qs in range(2):
                                    qsl = slice(qs * 512, (qs + 1) * 512)
                                    for i in range(2):
                                        nc.tensor.matmul(
                                            hpvs[i][0:65, qsl],
                                            v_sb[:, NJ - 1, hpair[i], :],
                                            hpts[(i,)][:, qsl],
                                            start=False, stop=True,
                                            skip_group_check=True)
                                held = None
                            if prev_pts is not None:
                                for qs in range(2):
                                    qsl = slice(qs * 512, (qs + 1) * 512)
                                    for i in range(2):
                                        mv = (prev_pts[(i,)][:, qsl]
                                              if (i,) in prev_pts
                                              else prev_pts[(i, qs)][:, :])
                                        nc.tensor.matmul(
                                            pvs[i][0:65, qsl],
                                            v_sb[:, j - 1, pair[i], :],
                                            mv, start=(j - 1 == 0),
                                            stop=False,
                                            skip_group_check=True)
                            prev_pts = pts
                            if pending is not None and j == 1:
                                emit_normalize(*pending)
                                pending = None
                        held = (pvs, pair, prev_pts)
                        pending = (pvs, pair)

                    hpvs, hpair, hpts = held
                    for qs in range(2):
                        qsl = slice(qs * 512, (qs + 1) * 512)
                        for i in range(2):
                            nc.tensor.matmul(
                                hpvs[i][0:65, qsl],
                                v_sb[:, NJ - 1, hpair[i], :],
                                hpts[(i,)][:, qsl], start=False, stop=True,
                                skip_group_check=True)
                    held = None
                    emit_normalize(*pending, last=True)

                    # ---- output projection + residual, split by e ----
                    # e = 0..2 (heads 0..5) are final long before the last
                    # pair's normalize; accumulate those partials (+ the
                    # residual, folded in) WHILE the normalize chain runs on
                    # DVE/GpSimd — useful work instead of junk warm-keeping.
                    # Only the e=3 matmul waits on heads 6/7's oT.
                    partial_sb = opool.tile([P, QR // P, E], f32, tag="prt",
                                            bufs=1)
                    for qc in range(QR // P):
                        pp = ps_c.tile([P, E], f32, tag="sc", name="pp")
                        for e in range(EC - 1):
                            nc.tensor.matmul(
                                pp[:], oT_sb[:, e, qc * P:(qc + 1) * P],
                                wo_sb[:, e, :], start=(e == 0),
                                stop=False, skip_group_check=True)
                        # residual folded in via identity matmul; partials
                        # evacuated on the idle ScalarE so the DVE queue is
                        # left entirely to the last pair's normalize chain
                        nc.tensor.matmul(
                            pp[:], ident_sb[:], xres_sb[:, qc, :],
                            start=False, stop=True, skip_group_check=True)
                        nc.scalar.copy(partial_sb[:, qc, :], pp[:])
                    for q2 in range(QR // P // 2):
                        po = ps_c.tile([P, 2, E], f32, tag="sc", name="po")
                        for k in range(2):
                            qc = 2 * q2 + k
                            nc.tensor.matmul(
                                po[:, k, :], oT_sb[:, EC - 1,
                                                   qc * P:(qc + 1) * P],
                                wo_sb[:, EC - 1, :], start=True, stop=True,
                                skip_group_check=True)
                        ot = opool.tile([P, 2, E], f32, tag="ot", name="ot")
                        nc.vector.tensor_add(
                            ot[:], po[:], partial_sb[:, 2 * q2:2 * q2 + 2, :])
                        for k in range(2):
                            qc = 2 * q2 + k
                            nc.sync.dma_start(
                                out=out_d[qc * P:(qc + 1) * P, :],
                                in_=ot[:, k, :])

    nc.compile()
    return nc


def _get_nc():
    if "nc" not in _CACHE:
        _CACHE["nc"] = _build_nc()
    return _CACHE["nc"]


def run_spmd(in_maps, **kw):
    from concourse.bass_utils import run_bass_kernel_spmd
    nc = _get_nc()
    return run_bass_kernel_spmd(nc, in_maps, list(range(8)), **kw)


def make_in_maps(x, Wq, bq, Wk, bk, Wv, bv, Wo, bo):
    import ml_dtypes
    bf = ml_dtypes.bfloat16
    x = np.asarray(x, dtype=np.float32)
    f32c = lambda a: np.ascontiguousarray(np.asarray(a, dtype=np.float32))
    bfc = lambda a: np.ascontiguousarray(
        np.asarray(a, dtype=np.float32).astype(bf))
    rearr = lambda wT: np.ascontiguousarray(
        wT.reshape(EC, P, E).transpose(1, 0, 2))
    wqT = rearr(bfc(np.asarray(Wq).T))
    wkT = rearr(bfc(np.asarray(Wk).T))
    wvT = rearr(bfc(np.asarray(Wv).T))
    woT = rearr(bfc(np.asarray(Wo).T))
    bq_r = f32c(np.asarray(bq).reshape(FC, P).T)
    bk_r = f32c(np.asarray(bk).reshape(FC, P).T)
    bv_a = f32c(bv)
    bo_a = np.asarray(bo, dtype=np.float32)
    xT = [np.ascontiguousarray(
        bfc(x[b].T).reshape(EC, P, S).transpose(1, 0, 2)) for b in range(B)]

    in_maps = []
    for c in range(8):
        b, r = c // 4, c % 4
        in_maps.append({
            "xT": xT[b],
            "xqT": np.ascontiguousarray(xT[b][:, :, r * QR:(r + 1) * QR]),
            # output bias folded into the residual tile (host-side, free)
            "xres": np.ascontiguousarray(
                bfc(x[b, r * QR:(r + 1) * QR] + bo_a)
                .reshape(QR // P, P, E).transpose(1, 0, 2)),
            "ident": np.eye(P, dtype=np.float32).astype(bf),
            "wqT": wqT, "wkT": wkT, "wvT": wvT, "woT": woT,
            "bq": bq_r, "bk": bk_r, "bv": bv_a,
        })
    return in_maps


def assemble(results):
    out = np.empty((B, S, E), dtype=np.float32)
    for c in range(8):
        b, r = c // 4, c % 4
        out[b, r * QR:(r + 1) * QR] = results[c]["out"]
    return out


def kernel(x, Wq, bq, Wk, bk, Wv, bv, Wo, bo):
    in_maps = make_in_maps(x, Wq, bq, Wk, bk, Wv, bv, Wo, bo)
    res = run_spmd(in_maps)
    return assemble(res.results)


# revision 35
# speedup vs baseline: 1.0072x; 1.0072x over previous
"""Trainium2 Bass kernel for an 8-head self-attention block (MHA).

Problem: x[2, 4096, 512], 8 heads x 64 dims, torch-Linear q/k/v/o projections,
softmax attention, residual:  out = softmax(q k^T / 8) v @ Wo^T + bo + x.

Sharding (8 NeuronCores, no collectives): core c handles batch b = c // 4 and
query rows (c % 4) * 1024 ... + 1024, for ALL heads.  K/V for the full
sequence are computed on every core of a batch group, so the output
projection is fully local to a core.

Engine-level structure (trace-driven; ACT exp is a hard ~300us floor, so the
whole kernel is built to keep ACT saturated while PE stays dense enough to
avoid the HAM half-clock throttle):

  - PV in "transposed" orientation: stationary = [V | 1] chunk [s=128, 65]
    (65-column LDWEIGHTS), moving = P~ [s=128, q=512] -> psum oT[65, q]
    accumulated over the 32 s-chunks; row 64 collects the softmax
    denominator.  No per-qc LDWEIGHTS stream, no PE transposes.
  - phase B: pair (0,1) attention at q=512 granularity, with ALL projection
    matmul groups (K, V, and the deferred 3/4 of Q) interleaved 2-per-chunk
    as PE filler between the exp-gated score matmuls.  Projections get
    their own 2-bank psum ring so they never queue behind an exp.
    Per-chunk PE work (~3.4us) > ACT work (2.9us): PE stays dense -> warm.
  - phase C: pairs (2,3), (4,5), (6,7) with q=1024 score tiles and exps
    (best ACT overhead that fits psum); measured 98% ACT occupancy.
  - normalization: one DVE copy stages psum rows 0:65 (freeing the psum
    accumulator), then gpsimd extracts/broadcasts the denominator row,
    DVE fast-reciprocal + multiply write bf16 oT (cross-partition-base
    DVE writes are legal and verified).
  - a dummy exp at t=0 preloads the ACT spline table during startup DMA;
    the first real exp lands ~10us in (startup emits only the f=0 chunks
    of Q and K that the first score matmul actually needs).
"""

import numpy as np

B = 2
S = 4096
E = 512
H = 8
D = 64
P = 128
EC = E // P          # 4 e-chunks
FC = E // P          # 4 f-chunks
NJ = S // P          # 32 s-chunks
QR = S // 4          # 1024 query rows per core
NQS = QR // 512      # 2 query strips of 512
NKS = S // 512       # 8 s-strips of 512

_CACHE = {}


def _build_nc():
    import concourse.bass as bass
    import concourse.tile as tile
    from concourse import bacc, mybir

    f32 = mybir.dt.float32
    bf16 = mybir.dt.bfloat16
    AFT = mybir.ActivationFunctionType
    Alu = mybir.AluOpType

    nc = bacc.Bacc("TRN2", target_bir_lowering=False, debug=False, num_devices=8)

    xT_d = nc.declare_dram_parameter("xT", [P, EC, S], bf16, isOutput=False)
    xqT_d = nc.declare_dram_parameter("xqT", [P, EC, QR], bf16, isOutput=False)
    xres_d = nc.declare_dram_parameter("xres", [P, QR // P, E], bf16,
                                       isOutput=False)
    ident_d = nc.declare_dram_parameter("ident", [P, P], bf16, isOutput=False)
    wqT_d = nc.declare_dram_parameter("wqT", [P, EC, E], bf16, isOutput=False)
    wkT_d = nc.declare_dram_parameter("wkT", [P, EC, E], bf16, isOutput=False)
    wvT_d = nc.declare_dram_parameter("wvT", [P, EC, E], bf16, isOutput=False)
    woT_d = nc.declare_dram_parameter("woT", [P, EC, E], bf16, isOutput=False)
    bq_d = nc.declare_dram_parameter("bq", [P, FC], f32, isOutput=False)
    bk_d = nc.declare_dram_parameter("bk", [P, FC], f32, isOutput=False)
    bv_d = nc.declare_dram_parameter("bv", [E], f32, isOutput=False)
    out_d = nc.declare_dram_parameter("out", [QR, E], f32, isOutput=True)

    with tile.TileContext(nc) as tc:
        with tc.tile_pool(name="const", bufs=1) as const, \
             tc.tile_pool(name="persist", bufs=1) as persist:

            wo_sb = const.tile([P, EC, E], bf16)
            bq_sb = const.tile([P, FC], f32)
            bk_sb = const.tile([P, FC], f32)
            bv_sb = const.tile([P, E], f32)
            xres_sb = const.tile([P, QR // P, E], bf16)
            ident_sb = const.tile([P, P], bf16)

            kT_sb = persist.tile([P, FC, S], bf16)           # 32 KB/p
            qT_sb = persist.tile([P, FC, QR], bf16)          # 8 KB/p
            v_sb = persist.tile([P, NJ, H, 65], bf16)        # 32.5 KB/p
            oT_sb = persist.tile([P, FC, QR], bf16)          # 8 KB/p

            ones_sb = const.tile([P, 64], f32)
            nc.vector.memset(v_sb[:, :, :, 64:65], 1.0)
            nc.vector.memset(ones_sb[0:1, :], 1.0)

            with tc.tile_pool(name="wpool", bufs=1) as wpool, \
                 tc.tile_pool(name="xtp", bufs=3) as xtp, \
                 tc.tile_pool(name="ptp", bufs=1) as ptp, \
                 tc.tile_pool(name="npool", bufs=1) as npool, \
                 tc.tile_pool(name="opool", bufs=2) as opool, \
                 tc.tile_pool(name="ps_pv", bufs=2, space="PSUM") as ps_pv:

                # dummy exp: preloads the ACT table set during startup DMA
                dum = npool.tile([P, 1], f32, tag="dum", bufs=1)
                nc.vector.memset(dum[:], 0.0)
                nc.scalar.activation(dum[:], dum[:], AFT.Exp)

                wq_sb = wpool.tile([P, EC, E], bf16)
                wk_sb = wpool.tile([P, EC, E], bf16)
                wv_sb = wpool.tile([P, EC, E], bf16)
                xq_t = [xtp.tile([P, EC, 512], bf16, tag="xq", bufs=2,
                                 name=f"xq{i}") for i in range(NQS)]
                # one DMA per tensor (per-DMA overhead ~1.5us dominates the
                # startup critical path at finer granularity)
                nc.sync.dma_start(out=xq_t[0][:], in_=xqT_d[:, :, 0:512])
                nc.sync.dma_start(out=wq_sb[:], in_=wqT_d[:])
                nc.sync.dma_start(out=bq_sb[:], in_=bq_d[:])
                # K path + strips on a second hardware DMA queue, in
                # parallel with the Q path above
                nc.scalar.dma_start(out=wk_sb[:], in_=wkT_d[:])
                nc.scalar.dma_start(out=bk_sb[:], in_=bk_d[:])
                xt_early = {}
                for _s in range(2):
                    _xt = xtp.tile([P, EC, 512], bf16, tag="xt",
                                   name=f"xte{_s}")
                    nc.scalar.dma_start(
                        out=_xt[:],
                        in_=xT_d[:, :, _s * 512:(_s + 1) * 512])
                    xt_early[_s] = _xt
                nc.sync.dma_start(out=xq_t[1][:], in_=xqT_d[:, :, 512:1024])
                nc.scalar.dma_start(out=wv_sb[:], in_=wvT_d[:])
                nc.scalar.dma_start(
                    out=bv_sb[:],
                    in_=bass.AP(tensor=bv_d, offset=0, ap=[[0, P], [1, E]]))

                xt_tiles = {}

                def dma_strip(s):
                    if s in xt_early:
                        xt_tiles[s] = xt_early[s]
                        return
                    xt = xtp.tile([P, EC, 512], bf16, tag="xt")
                    ssl = slice(s * 512, (s + 1) * 512)
                    nc.sync.dma_start(out=xt[:], in_=xT_d[:, :, ssl])
                    xt_tiles[s] = xt

                def emit_normalize(pvs, pair, last=False, bc_pool=None):
                    # oT[0:64] = psum_rows_0:63 * (1 / psum_row_64), per head
                    for i, h in enumerate(pair):
                        fc = h // 2
                        fr = (h % 2) * 64
                        den = npool.tile([P, QR], f32, tag="den", bufs=2)
                        if last:
                            # no next pair waits on these psum banks: read
                            # psum directly; cross-base psum->sbuf DVE copy
                            # (proven by the head-7 multiply) extracts the
                            # denominator row in one step
                            nc.vector.tensor_copy(den[0:1, :],
                                                  pvs[i][64:65, 0:QR])
                            stg = pvs[i]
                        else:
                            stg = npool.tile([P, QR], f32, tag="stg", bufs=2)
                            nc.vector.tensor_copy(stg[0:65, :],
                                                  pvs[i][0:65, 0:QR])
                            nc.vector.tensor_copy(den[0:1, :],
                                                  stg[64:65, :])
                        rc = npool.tile([P, QR], f32, tag="rc", bufs=1)
                        if bc_pool is not None:
                            # tail: K=1 ones-row matmul broadcasts on the idle
                            # PE in ~0.5us (gpsimd bcast measures 1.8-5.2us)
                            bcp = bc_pool.tile([P, QR], f32, tag="sc",
                                               name="bcp")
                            for qs in range(2):
                                qsl = slice(qs * 512, (qs + 1) * 512)
                                nc.tensor.matmul(
                                    bcp[0:64, qsl], ones_sb[0:1, 0:64],
                                    den[0:1, qsl], start=True, stop=True,
                                    skip_group_check=True)
                            nc.vector.reciprocal_approx_fast(
                                rc[0:64, :], bcp[0:64, 0:QR])
                        else:
                            bc = npool.tile([P, QR], f32, tag="bc", bufs=1)
                            nc.gpsimd.partition_broadcast(
                                bc[0:64, :], den[0:1, :], channels=64)
                            nc.vector.reciprocal_approx_fast(
                                rc[0:64, :], bc[0:64, :])
                        nc.vector.tensor_mul(
                            oT_sb[fr:fr + 64, fc, :], stg[0:64, :] if not last
                            else pvs[i][0:64, 0:QR], rc[0:64, :])

                # ---------- phase B ----------
                with tc.tile_pool(name="ps_b", bufs=2, space="PSUM") as ps_b:

                    def emit_pq(f, qs):
                        qsl = slice(qs * 512, (qs + 1) * 512)
                        pq = ps_b.tile([P, 512], f32, tag="pj", name="pq")
                        for e in range(EC):
                            nc.tensor.matmul(
                                pq[:], wq_sb[:, e, f * P:(f + 1) * P],
                                xq_t[qs][:, e, :], start=(e == 0),
                                stop=(e == EC - 1), skip_group_check=True)
                        nc.vector.tensor_scalar(
                            qT_sb[:, f, qsl], pq[:], bq_sb[:, f:f + 1],
                            float(1.0 / np.sqrt(D)), Alu.add, Alu.mult)

                    def emit_pk(s, f):
                        ssl = slice(s * 512, (s + 1) * 512)
                        pk = ps_b.tile([P, 512], f32, tag="pj", name="pk")
                        for e in range(EC):
                            nc.tensor.matmul(
                                pk[:], wk_sb[:, e, f * P:(f + 1) * P],
                                xt_tiles[s][:, e, :], start=(e == 0),
                                stop=(e == EC - 1), skip_group_check=True)
                        nc.vector.tensor_scalar_add(
                            kT_sb[:, f, ssl], pk[:], bk_sb[:, f:f + 1])

                    def emit_v(s, k):
                        j = s * 4 + k
                        pvx = ps_b.tile([P, E], f32, tag="pj", name="pvx")
                        for e in range(EC):
                            nc.tensor.matmul(
                                pvx[:], xt_tiles[s][:, e, k * P:(k + 1) * P],
                                wv_sb[:, e, :], start=(e == 0),
                                stop=(e == EC - 1), skip_group_check=True)
                        pv_v = pvx[:].rearrange("p (h d) -> p h d", h=H)
                        bv_v = bv_sb[:].rearrange("p (h d) -> p h d", h=H)
                        nc.vector.tensor_add(v_sb[:, j, :, 0:64], pv_v[:],
                                             bv_v[:])

                    # filler schedule: (deadline_slot, earliest_slot, fn, args)
                    # pk(s,*) must land inside strip s's xt-tile residency
                    # window [4(s-2), 4s+3] (ring bufs=3); deadlines spread
                    # the per-strip groups ~2 per chunk slot.
                    fillers = []
                    for s in range(1, NKS):
                        fillers.append([4 * s - 1, 4 * (s - 2), emit_pk, (s, 0)])
                    for s in range(NKS):
                        for k in range(4):
                            fillers.append([4 * s + k, 4 * (s - 2), emit_v,
                                            (s, k)])
                        for f in range(1, FC):
                            fillers.append([4 * s + f, 4 * (s - 2), emit_pk,
                                            (s, f)])
                    for f in range(1, FC):
                        for qs in range(NQS):
                            # f=3 Q chunks spill into the bridge's PE slack
                            dl = (NJ + qs) if f == 3 else 8 * f + qs
                            fillers.append([dl, 0, emit_pq, (f, qs)])
                    for fl in fillers:
                        if fl[2] is emit_pk and fl[3] == (NKS - 1, FC - 1):
                            fl[0] = NJ + 2   # pk(7,3) -> bridge slot 2
                    fillers.sort(key=lambda x: x[0])

                    def pop_fillers(j, target=2):
                        n = 0
                        while fillers and fillers[0][0] <= j:
                            _, _, fn, args = fillers.pop(0)
                            fn(*args)
                            n += 1
                        while n < target:
                            idx = next((k for k, fl in enumerate(fillers)
                                        if fl[1] <= j), None)
                            if idx is None:
                                break
                            _, _, fn, args = fillers.pop(idx)
                            fn(*args)
                            n += 1

                    # warm the PE during the startup DMA window: ~26 junk
                    # matmuls on (uninitialized) SBUF, results never read.
                    # HAM needs ~3.4us of sustained PE activity to lift the
                    # half-clock throttle; without this every startup matmul
                    # runs at 1.2 GHz.
                    for w in range(7):
                        jk = ps_b.tile([P, 512], f32, tag="pj", name="jk")
                        nc.tensor.matmul(
                            jk[:], kT_sb[:, 0, 0:P], kT_sb[:, 0, 0:512],
                            start=True, stop=True, skip_group_check=True)

                    # startup compute: just what score chunk j=0 needs
                    dma_strip(0)
                    dma_strip(1)
                    emit_pq(0, 0)
                    emit_pk(0, 0)
                    emit_pq(0, 1)

                    pv01 = [ps_pv.tile([P, QR], f32, tag="pv", name=f"pv0{i}")
                            for i in range(2)]
                    prev_pts = None
                    for j in range(NJ):
                        if j % 4 == 0 and j // 4 + 2 < NKS:
                            dma_strip(j // 4 + 2)
                        pts = {}
                        for qs in range(2):
                            qsl = slice(qs * 512, (qs + 1) * 512)
                            scs = []
                            for i in range(2):
                                fr = i * 64
                                sc = ps_b.tile([P, 512], f32, tag="sc",
                                               name="scb")
                                nc.tensor.matmul(
                                    sc[:],
                                    kT_sb[fr:fr + 64, 0, j * P:(j + 1) * P],
                                    qT_sb[fr:fr + 64, 0, qsl],
                                    start=True, stop=True,
                                    skip_group_check=True)
                                scs.append(sc)
                            for i in range(2):
                                pt = ptp.tile([P, 512], bf16, tag="ptb",
                                              bufs=8)
                                nc.scalar.activation(pt[:], scs[i][:], AFT.Exp)
                                pts[(i, qs)] = pt
                            if qs == 0 and prev_pts is not None:
                                for q2 in range(2):
                                    q2l = slice(q2 * 512, (q2 + 1) * 512)
                                    for i in range(2):
                                        nc.tensor.matmul(
                                            pv01[i][0:65, q2l],
                                            v_sb[:, j - 1, i, :],
                                            prev_pts[(i, q2)][:, :],
                                            start=(j - 1 == 0),
                                            stop=(j - 1 == NJ - 1),
                                            skip_group_check=True)
                        pop_fillers(j)
                        prev_pts = pts
                    held01 = prev_pts
                    pending = (pv01, (0, 1))

                    # bridge: pair (2,3)'s first chunks at q=512 inside the
                    # B psum scope, so neither PE nor ACT idles across the
                    # B->C pool handoff (an idle window here re-throttles
                    # the PE to half clock for the whole next pair).
                    NBR = 3
                    pv23 = [ps_pv.tile([P, QR], f32, tag="pv",
                                       name=f"pv23{i}") for i in range(2)]
                    prev_pts23 = None
                    for j in range(NBR):
                        pts = {}
                        for qs in range(2):
                            qsl = slice(qs * 512, (qs + 1) * 512)
                            scs = []
                            for i in range(2):
                                fr = i * 64
                                sc = ps_b.tile([P, 512], f32, tag="sc",
                                               name="scb2")
                                nc.tensor.matmul(
                                    sc[:],
                                    kT_sb[fr:fr + 64, 1, j * P:(j + 1) * P],
                                    qT_sb[fr:fr + 64, 1, qsl],
                                    start=True, stop=True,
                                    skip_group_check=True)
                                scs.append(sc)
                            for i in range(2):
                                pt = ptp.tile([P, 512], bf16, tag="ptb",
                                              bufs=8)
                                nc.scalar.activation(pt[:], scs[i][:], AFT.Exp)
                                pts[(i, qs)] = pt
                            if qs == 0 and held01 is not None:
                                for q2 in range(2):
                                    q2l = slice(q2 * 512, (q2 + 1) * 512)
                                    for i in range(2):
                                        nc.tensor.matmul(
                                            pv01[i][0:65, q2l],
                                            v_sb[:, NJ - 1, i, :],
                                            held01[(i, q2)][:, :],
                                            start=False, stop=True,
                                            skip_group_check=True)
                                held01 = None
                            if qs == 0 and prev_pts23 is not None:
                                for q2 in range(2):
                                    q2l = slice(q2 * 512, (q2 + 1) * 512)
                                    for i in range(2):
                                        nc.tensor.matmul(
                                            pv23[i][0:65, q2l],
                                            v_sb[:, j - 1, 2 + i, :],
                                            prev_pts23[(i, q2)][:, :],
                                            start=(j - 1 == 0), stop=False,
                                            skip_group_check=True)
                        pop_fillers(NJ + j, target=0)
                        prev_pts23 = pts
                        if j == 1:
                            emit_normalize(*pending)
                            pending = None
                    assert not fillers, [f[0] for f in fillers]

                # tail-only data, off the startup critical path
                nc.sync.dma_start(out=wo_sb[:], in_=woT_d[:])
                nc.sync.dma_start(out=xres_sb[:], in_=xres_d[:])
                nc.sync.dma_start(out=ident_sb[:], in_=ident_d[:])

                # ---------- phase C ----------
                with tc.tile_pool(name="ps_c", bufs=2, space="PSUM") as ps_c:
                    held = None   # previous pair's final PV, deferred so the
                    # next pair's first scores reach the PE queue first and
                    # ACT never gaps across the pair boundary
                    for pi in range(1, 4):
                        pair = (2 * pi, 2 * pi + 1)
                        fc = pi
                        if pi == 1:
                            # resume pair (2,3): bridge already ran j < NBR
                            pvs = pv23
                            prev_pts = prev_pts23
                            jstart = NBR
                        else:
                            pvs = [ps_pv.tile([P, QR], f32, tag="pv",
                                              name=f"pv{pi}{i}")
                                   for i in range(2)]
                            prev_pts = None
                            jstart = 0
                        for j in range(jstart, NJ):
                            jsl = slice(j * P, (j + 1) * P)
                            scs = []
                            for i in range(2):
                                scs.append(ps_c.tile([P, QR], f32, tag="sc",
                                                     name="scc"))
                            for qs in range(2):
                                qsl = slice(qs * 512, (qs + 1) * 512)
                                for i in range(2):
                                    fr = i * 64
                                    nc.tensor.matmul(
                                        scs[i][:, qsl],
                                        kT_sb[fr:fr + 64, fc, jsl],
                                        qT_sb[fr:fr + 64, fc, qsl],
                                        start=True, stop=True,
                                        skip_group_check=True)
                            pts = {}
                            for i in range(2):
                                pt = ptp.tile([P, QR], bf16, tag="ptc", bufs=5)
                                nc.scalar.activation(pt[:], scs[i][:], AFT.Exp)
                                pts[(i,)] = pt
                            if held is not None:
                                hpvs, hpair, hpts = held
                                for qs in range(2):
                                    qsl = slice(qs * 512, (qs + 1) * 512)
                                    for i in range(2):
                                        nc.tensor.matmul(
                                            hpvs[i][0:65, qsl],
                                            v_sb[:, NJ - 1, hpair[i], :],
                                            hpts[(i,)][:, qsl],
                                            start=False, stop=True,
                                            skip_group_check=True)
                                held = None
                            if prev_pts is not None:
                                for qs in range(2):
                                    qsl = slice(qs * 512, (qs + 1) * 512)
                                    for i in range(2):
                                        mv = (prev_pts[(i,)][:, qsl]
                                              if (i,) in prev_pts
                                              else prev_pts[(i, qs)][:, :])
                                        nc.tensor.matmul(
                                            pvs[i][0:65, qsl],
                                            v_sb[:, j - 1, pair[i], :],
                                            mv, start=(j - 1 == 0),
                                            stop=False,
                                            skip_group_check=True)
                            prev_pts = pts
                            if pending is not None and j == 1:
                                emit_normalize(*pending)
                                pending = None
                        held = (pvs, pair, prev_pts)
                        pending = (pvs, pair)

                    hpvs, hpair, hpts = held
                    for qs in range(2):
                        qsl = slice(qs * 512, (qs + 1) * 512)
                        for i in range(2):
                            nc.tensor.matmul(
                                hpvs[i][0:65, qsl],
                                v_sb[:, NJ - 1, hpair[i], :],
                                hpts[(i,)][:, qsl], start=False, stop=True,
                                skip_group_check=True)
                    held = None
                    emit_normalize(*pending, last=True)

                    # ---- output projection + residual, split by e ----
                    # e = 0..2 (heads 0..5) are final long before the last
                    # pair's normalize; accumulate those partials (+ the
                    # residual, folded in) WHILE the normalize chain runs on
                    # DVE/GpSimd — useful work instead of junk warm-keeping.
                    # Only the e=3 matmul waits on heads 6/7's oT.
                    partial_sb = opool.tile([P, QR // P, E], f32, tag="prt",
                                            bufs=1)
                    for qc in range(QR // P):
                        pp = ps_c.tile([P, E], f32, tag="sc", name="pp")
                        for e in range(EC - 1):
                            nc.tensor.matmul(
                                pp[:], oT_sb[:, e, qc * P:(qc + 1) * P],
                                wo_sb[:, e, :], start=(e == 0),
                                stop=False, skip_group_check=True)
                        # residual folded in via identity matmul; partials
                        # evacuated on the idle ScalarE so the DVE queue is
                        # left entirely to the last pair's normalize chain
                        nc.tensor.matmul(
                            pp[:], ident_sb[:], xres_sb[:, qc, :],
                            start=False, stop=True, skip_group_check=True)
                        nc.scalar.copy(partial_sb[:, qc, :], pp[:])
                    for q2 in range(QR // P // 2):
                        po = ps_c.tile([P, 2, E], f32, tag="sc", name="po")
                        for k in range(2):
                            qc = 2 * q2 + k
                            nc.tensor.matmul(
                                po[:, k, :], oT_sb[:, EC - 1,
                                                   qc * P:(qc + 1) * P],
                                wo_sb[:, EC - 1, :], start=True, stop=True,
                                skip_group_check=True)
                        ot = opool.tile([P, 2, E], f32, tag="ot", name="ot")
                        nc.vector.tensor_add(
                            ot[:], po[:], partial_sb[:, 2 * q2:2 * q2 + 2, :])
                        for k in range(2):
                            qc = 2 * q2 + k
                            nc.sync.dma_start(
                                out=out_d[qc * P:(qc + 1) * P, :],
                                in_=ot[:, k, :])

    nc.compile()
    return nc


def _get_nc():
    if "nc" not in _CACHE:
        _CACHE["nc"] = _build_nc()
    return _CACHE["nc"]


def run_spmd(in_maps, **kw):
    from concourse.bass_utils import run_bass_kernel_spmd
    nc = _get_nc()
    return run_bass_kernel_spmd(nc, in_maps, list(range(8)), **kw)


def make_in_maps(x, Wq, bq, Wk, bk, Wv, bv, Wo, bo):
    import ml_dtypes
    bf = ml_dtypes.bfloat16
    x = np.asarray(x, dtype=np.float32)
    f32c = lambda a: np.ascontiguousarray(np.asarray(a, dtype=np.float32))
    bfc = lambda a: np.ascontiguousarray(
        np.asarray(a, dtype=np.float32).astype(bf))
    rearr = lambda wT: np.ascontiguousarray(
        wT.reshape(EC, P, E).transpose(1, 0, 2))
    wqT = rearr(bfc(np.asarray(Wq).T))
    wkT = rearr(bfc(np.asarray(Wk).T))
    wvT = rearr(bfc(np.asarray(Wv).T))
    woT = rearr(bfc(np.asarray(Wo).T))
    bq_r = f32c(np.asarray(bq).reshape(FC, P).T)
    bk_r = f32c(np.asarray(bk).reshape(FC, P).T)
    bv_a = f32c(bv)
    bo_a = np.asarray(bo, dtype=np.float32)
    xT = [np.ascontiguousarray(
        bfc(x[b].T).reshape(EC, P, S).transpose(1, 0, 2)) for b in range(B)]

    in_maps = []
    for c in range(8):
        b, r = c // 4, c % 4
        in_maps.append({
            "xT": xT[b],
            "xqT": np.ascontiguousarray(xT[b][:, :, r * QR:(r + 1) * QR]),
            # output bias folded into the residual tile (host-side, free)
            "xres": np.ascontiguousarray(
                bfc(x[b, r * QR:(r + 1) * QR] + bo_a)
                .reshape(QR // P, P, E).transpose(1, 0, 2)),
            "ident": np.eye(P, dtype=np.float32).astype(bf),
            "wqT": wqT, "wkT": wkT, "wvT": wvT, "woT": woT,
            "bq": bq_r, "bk": bk_r, "bv": bv_a,
        })
    return in_maps


def assemble(results):
    out = np.empty((B, S, E), dtype=np.float32)
    for c in range(8):
        b, r = c // 4, c % 4
        out[b, r * QR:(r + 1) * QR] = results[c]["out"]
    return out


def kernel(x, Wq, bq, Wk, bk, Wv, bv, Wo, bo):
    in_maps = make_in_maps(x, Wq, bq, Wk, bk, Wv, bv, Wo, bo)
    res = run_spmd(in_maps)
    return assemble(res.results)
